# revision 1
# baseline (speedup 1.0000x reference)
"""Causal self-attention kernel for Trainium2, 8-way sharded.

Problem: B=2, T=2048, C=1024, NH=16, hd=64. fp32 in/out.

Sharding: core = (batch b, head-group g of 4 heads). Each core computes its
4 heads' attention for its batch plus the partial output projection
y_local @ Wo[g*256:(g+1)*256, :]; the host sums the 4 partials per batch
(biases bv/bo are folded in exactly via a host-side correction row).

Device design notes:
  - Projection / score matmuls in float32r (full PE rate, ~1e-4 err);
    P^T and V in bf16 (softmax weights tolerate it; ~2e-3 total err).
  - Everything stays transposed (qT/kT/S^T/P^T/y^T): no on-chip transposes.
    Scores: S^T[tk,tq] = kT.T @ qT as K=64 row-tiled pairs (two heads run
    concurrently in different PE row groups).
  - Softmax without max-subtraction (scores are O(1), exp-safe in fp32):
    P^T = exp(S^T/8) on ScalarE, straight PSUM -> SBUF bf16.
  - Causal masking of diagonal-band tiles via GPSIMD affine_select.
  - PV: y^T_aug[72,tq] += V_aug[tk,72].T @ P^T with 8 appended ones columns
    in V_aug -> softmax denominators appear in PSUM rows 64-71; a K=1
    selector matmul moves each window's denominator onto its own row of a
    PSUM "denominator board" (rows 0-7), enabling one batched 8-lane
    reciprocal per pair; K=8 selector matmuls broadcast the reciprocals.
  - The exp stream is the second-largest engine load (~90us on ScalarE vs
    ~110us of matmuls), so the program is emitted as ONE fused loop that
    paces S^T slots at exp speed and fills the PE gaps with lagged PV
    matmuls and projection bursts (v / qk of the second head-pair) at
    instruction granularity.  PE order never depends on a later PE op.
"""
import contextlib

import ml_dtypes
import numpy as np

import concourse.bass as bass
import concourse.tile as tile
from concourse import bacc, mybir
from concourse import bass_utils

bass_utils.upload_artifacts = lambda tmpdir: "local://skipped"

B, T, C = 2, 2048, 1024
NH, HD = 16, 64
NHL = 4            # heads per core
CLOC = NHL * HD    # 256 local channels
NCH = C // 128     # 8 contraction chunks
TQW = 512          # tq window
NW = T // TQW      # 4 windows
NTT = T // 128     # 16 t-tiles / tk-chunks
VSTR = HD + 8      # 72: v cols per head + 8 ones cols (denoms at rows 64-71)
LAG = 2            # PV trails S^T by this many chunk-groups
F32R = mybir.dt.float32r
F32 = mybir.dt.float32
BF16 = mybir.dt.bfloat16

_cache = {}


def _build():
    nc = bacc.Bacc("TRN2", target_bir_lowering=False, debug=False, num_devices=8)

    xt_ap = nc.dram_tensor("xt", [128, NCH * T], F32R, kind="ExternalInput").ap()
    wq_ap = nc.dram_tensor("wq", [128, 2 * NCH * 128], F32R, kind="ExternalInput").ap()
    wk_ap = nc.dram_tensor("wk", [128, 2 * NCH * 128], F32R, kind="ExternalInput").ap()
    wv_ap = nc.dram_tensor("wv", [128, NCH * CLOC], F32R, kind="ExternalInput").ap()
    wo_ap = nc.dram_tensor("wo", [128, 2 * C], F32R, kind="ExternalInput").ap()
    bq_ap = nc.dram_tensor("bq", [2, 128, 1], F32, kind="ExternalInput").ap()
    bk_ap = nc.dram_tensor("bk", [2, 128, 1], F32, kind="ExternalInput").ap()
    ones_ap = nc.dram_tensor("ones", [128, NTT, NHL, 8], BF16, kind="ExternalInput").ap()
    sels_ap = nc.dram_tensor("sels", [128, 512], F32, kind="ExternalInput").ap()
    selc_ap = nc.dram_tensor("selc", [128, 64], F32R, kind="ExternalInput").ap()
    tri_ap = nc.dram_tensor("tri", [128, 128], BF16, kind="ExternalInput").ap()
    out_ap = nc.dram_tensor("out", [T, C], F32, kind="ExternalOutput").ap()

    with tile.TileContext(nc) as tc, contextlib.ExitStack() as ctx:
        sb = ctx.enter_context(tc.tile_pool(name="sb", bufs=1))
        r_pool = ctx.enter_context(tc.tile_pool(name="rp", bufs=1))
        ost_pool = ctx.enter_context(tc.tile_pool(name="ost", bufs=2))
        pt_pool = ctx.enter_context(tc.tile_pool(name="ptp", bufs=10))
        ps = ctx.enter_context(tc.tile_pool(name="ps", bufs=1, space="PSUM"))

        # ---- persistent SBUF tensors ----
        wqs = sb.tile([128, 2 * NCH * 128], F32R, tag="wqs")
        wks = sb.tile([128, 2 * NCH * 128], F32R, tag="wks")
        wvs = sb.tile([128, NCH * CLOC], F32R, tag="wvs")
        wos = sb.tile([128, 2 * C], F32R, tag="wos")
        xts = sb.tile([128, NCH * T], F32R, tag="xts")
        qts = [sb.tile([128, T], F32R, tag=f"qt{p}", name=f"qt{p}") for p in range(2)]
        kts = [sb.tile([128, T], F32R, tag=f"kt{p}", name=f"kt{p}") for p in range(2)]
        vna = sb.tile([128, NTT * NHL * VSTR], BF16, tag="vna")
        yts = [sb.tile([128, T], F32R, tag=f"yt{p}", name=f"yt{p}") for p in range(2)]
        bqs = [sb.tile([128, 1], F32, tag=f"bq{p}", name=f"bqs{p}") for p in range(2)]
        bks = [sb.tile([128, 1], F32, tag=f"bk{p}", name=f"bks{p}") for p in range(2)]
        sels = sb.tile([128, 512], F32, tag="sels")
        selc = sb.tile([128, 64], F32R, tag="selc")
        dsb = sb.tile([128, TQW], F32, tag="dsb")
        tri = sb.tile([128, 128], BF16, tag="tri")
        rcoll = sb.tile([128, TQW], F32, tag="rcoll")

        # ---- input DMAs: spread over the 3 DMA-capable queues; the first
        # matmuls need wq + xt chunk 0, so those go first on their queues.
        nc.sync.dma_start(wqs[:], wq_ap[:])
        nc.gpsimd.dma_start(xts[:, 0:2 * T], xt_ap[:, 0:2 * T])
        nc.scalar.dma_start(xts[:, 2 * T:4 * T], xt_ap[:, 2 * T:4 * T])
        nc.sync.dma_start(wks[:], wk_ap[:])
        nc.gpsimd.dma_start(xts[:, 4 * T:6 * T], xt_ap[:, 4 * T:6 * T])
        nc.scalar.dma_start(xts[:, 6 * T:8 * T], xt_ap[:, 6 * T:8 * T])
        for p in range(2):
            nc.sync.dma_start(bqs[p][:], bq_ap[p])
            nc.sync.dma_start(bks[p][:], bk_ap[p])
        nc.sync.dma_start(sels[:], sels_ap[:])
        nc.sync.dma_start(selc[:], selc_ap[:])
        nc.sync.dma_start(tri[:], tri_ap[:])
        vna4 = vna[:].rearrange("p (t h v) -> p t h v", t=NTT, h=NHL)
        nc.gpsimd.dma_start(vna4[:, :, :, HD:HD + 8], ones_ap[:])
        nc.scalar.dma_start(wvs[:], wv_ap[:])
        nc.scalar.dma_start(wos[:], wo_ap[:])

        pt_tiles = {}
        D_tiles = {}

        # ---------- emission primitives ----------
        def warm():
            wtile = sb.tile([128, 640], BF16, tag="warm")
            wjunk = sb.tile([128, 8], F32, tag="wjunk")
            nc.vector.memset(wtile[:], 0.0)
            wp = ps.tile([128, TQW], F32, tag="work", bufs=1, name="warm_ps")
            for i in range(40):
                nc.tensor.matmul(wp[:], wtile[:, 0:128], wtile[:, 128:640],
                                 start=True, stop=True)
            nc.vector.tensor_copy(wjunk[:], wp[:, 0:8])

        def qk_window(p, ty, w):
            wsb, dst, bias = ((wqs, qts[p], bqs[p]), (wks, kts[p], bks[p]))[ty]
            tag = "st" if p == 0 else "work"
            acc = ps.tile([128, TQW], F32, tag=tag, bufs=2 if p == 0 else 1,
                          name=f"qk{p}{ty}{w}")
            for c in range(NCH):
                nc.tensor.matmul(
                    acc[:], wsb[:, (p * NCH + c) * 128:(p * NCH + c + 1) * 128],
                    xts[:, c * T + w * TQW: c * T + w * TQW + TQW],
                    start=(c == 0), stop=(c == NCH - 1))
            nc.vector.tensor_scalar_add(dst[:, w * TQW:(w + 1) * TQW],
                                        acc[:], bias[:])

        def v_tile(tt):
            acc = ps.tile([128, CLOC], F32, tag="work", bufs=1, name=f"v{tt}")
            for c in range(NCH):
                nc.tensor.matmul(acc[:], xts[:, c * T + tt * 128: c * T + tt * 128 + 128],
                                 wvs[:, c * CLOC:(c + 1) * CLOC],
                                 start=(c == 0), stop=(c == NCH - 1))
            base = tt * NHL * VSTR
            dst = vna[:, base:base + NHL * VSTR].rearrange("p (h d) -> p h d", h=NHL)
            nc.vector.tensor_copy(dst[:, :, 0:HD],
                                  acc[:].rearrange("p (h d) -> p h d", h=NHL))

        def st_slot(p, w, g, h):
            qt, kt = qts[p], kts[p]
            nchunks = 4 * (w + 1)
            c0 = 2 * g
            st = ps.tile([128, 1024], F32, tag="st", bufs=2,
                         name=f"st{p}{w}{g}{h}")
            for j in range(2):
                c = c0 + j
                nc.tensor.matmul(
                    st[:, j * TQW:(j + 1) * TQW],
                    kt[h * 64:(h + 1) * 64, c * 128:(c + 1) * 128],
                    qt[h * 64:(h + 1) * 64, w * TQW:(w + 1) * TQW],
                    start=True, stop=True)
            pt = pt_pool.tile([128, 1024], BF16, tag="pt", name=f"pt{p}{w}{g}{h}")
            nc.scalar.activation(pt[:], st[:], mybir.ActivationFunctionType.Exp,
                                 scale=0.125)
            if c0 + 1 >= nchunks - 4:
                # causal mask: chunk c covers tq in [0,512) of this window,
                # diag 128-block at cols [128*jp, 128*jp+128), left of it = 0
                for j in range(2):
                    jp = (c0 + j) - 4 * w
                    if jp > 0:
                        nc.gpsimd.memset(pt[:, j * TQW: j * TQW + 128 * jp], 0.0)
                    dslc = pt[:, j * TQW + 128 * jp: j * TQW + 128 * jp + 128]
                    nc.vector.tensor_mul(dslc, dslc, tri[:])
            pt_tiles[(p, w, g, h)] = pt

        def pv_group(p, w, g, h, accs):
            nchunks = 4 * (w + 1)
            c0 = 2 * g
            pt = pt_tiles.pop((p, w, g, h))
            for j in range(2):
                c = c0 + j
                vbase = c * NHL * VSTR + (2 * p + h) * VSTR
                nc.tensor.matmul(
                    accs[h][0:VSTR, :],
                    vna[:, vbase:vbase + VSTR],
                    pt[:, j * TQW:(j + 1) * TQW],
                    start=(c0 == 0 and j == 0),
                    stop=(c0 == nchunks - 2 and j == 1))

        def pv_tail(p, w, h, accs):
            yt = yts[p]
            D = D_tiles[p]
            nc.vector.tensor_copy(yt[h * 64:(h + 1) * 64, w * TQW:(w + 1) * TQW],
                                  accs[h][0:HD, :])
            i = 2 * w + h
            rstage = r_pool.tile([128, TQW], F32R, tag="rstage", bufs=2,
                                 name=f"rst{p}{w}{h}")
            nc.vector.tensor_copy(rstage[HD:HD + 1, :], accs[h][HD:HD + 1, :])
            nc.tensor.matmul(D[0:8, :], selc[HD:HD + 1, 8 * i:8 * i + 8],
                             rstage[HD:HD + 1, :], start=False,
                             stop=(i == 7), skip_group_check=True)

        def norm_item(p):
            D = D_tiles.pop(p)
            nc.vector.tensor_copy(dsb[0:8, :], D[0:8, :])
            nc.vector.reciprocal(rcoll[0:8, :], dsb[0:8, :])
            for w in range(NW):
                R = ps.tile([128, TQW], F32, tag="work", bufs=1, name=f"R{p}{w}")
                nc.tensor.matmul(R[:], sels[0:8, w * 128:(w + 1) * 128],
                                 rcoll[0:8, :], start=True, stop=True)
                for h in range(2):
                    yslc = yts[p][h * 64:(h + 1) * 64, w * TQW:(w + 1) * TQW]
                    nc.vector.tensor_mul(yslc, yslc, R[h * 64:(h + 1) * 64, :])

        def out_item():
            for tt in range(NTT):
                po = ps.tile([128, 1024], F32, tag="st", bufs=2, name=f"po{tt}")
                for nh in range(2):
                    for cc in range(2):
                        nc.tensor.matmul(po[:, nh * TQW:(nh + 1) * TQW],
                                         yts[cc][:, tt * 128:(tt + 1) * 128],
                                         wos[:, cc * C + nh * TQW: cc * C + nh * TQW + TQW],
                                         start=(cc == 0), stop=(cc == 1))
                for nh in range(2):
                    ost = ost_pool.tile([128, TQW], F32, tag="ost", name=f"o{tt}{nh}")
                    if (2 * tt + nh) % 2 == 0:
                        nc.vector.tensor_copy(ost[:], po[:, nh * TQW:(nh + 1) * TQW])
                    else:
                        nc.scalar.copy(ost[:], po[:, nh * TQW:(nh + 1) * TQW])
                    deng = (nc.sync, nc.gpsimd, nc.scalar)[(2 * tt + nh) % 3]
                    deng.dma_start(
                        out_ap[tt * 128:(tt + 1) * 128, nh * TQW:(nh + 1) * TQW],
                        ost[:])

        # ---------- fused schedule ----------
        # filler: list of (kind, id, fn) emitting ~1-2us of dense PE work
        filler = []
        for tt in range(NTT):
            filler.append(("v", tt, lambda tt=tt: v_tile(tt)))
        for ty in range(2):
            for w in range(NW):
                filler.append(("qk1", None, lambda ty=ty, w=w: qk_window(1, ty, w)))
        filler_pe = {"v": 0.95, "qk1": 1.84}     # us of PE work per burst

        state = {"deficit": 0.0}  # ACT-emitted minus PE-emitted (us)

        def pull_filler(min_deficit=0.0, need_v=None, need_qk1=False):
            while filler:
                kind, ident, fn = filler[0]
                forced = (need_v is not None and kind == "v" and ident <= need_v) \
                         or (need_qk1 and kind == "qk1")
                if not forced and state["deficit"] < min_deficit:
                    return
                filler.pop(0)
                fn()
                state["deficit"] -= filler_pe[kind]
                if forced:
                    continue

        warm()
        for ty in range(2):
            for w in range(NW):
                qk_window(0, ty, w)

        for p in range(2):
            if p == 1:
                # ensure pair-1 q/k projections are in the PE stream first
                pull_filler(need_v=NTT, need_qk1=True)
            D = ps.tile([128, TQW], F32, tag="D", bufs=1, name=f"D{p}")
            nc.vector.memset(D[0:8, :], 0.0)
            D_tiles[p] = D
            for w in range(NW):
                ngroups = 2 * (w + 1)
                accs = [ps.tile([128, TQW], F32, tag=f"acc{h}", bufs=1,
                                name=f"acc{p}{w}{h}") for h in range(2)]
                for g in range(ngroups + LAG):
                    if g < ngroups:
                        # PV of group g needs v tiles for chunks 2g, 2g+1
                        pull_filler(need_v=2 * g + 1)
                        st_slot(p, w, g, 0)
                        st_slot(p, w, g, 1)
                        state["deficit"] += 2.3 - 0.94
                    if g >= LAG:
                        gg = g - LAG
                        pv_group(p, w, gg, 0, accs)
                        pv_group(p, w, gg, 1, accs)
                        state["deficit"] -= 0.86
                    pull_filler(min_deficit=1.0)
                for h in range(2):
                    pv_tail(p, w, h, accs)
            norm_item(p)
        # any leftover filler (shouldn't be much)
        pull_filler(need_v=NTT, need_qk1=True)
        out_item()

    nc.compile()
    return nc


def _sels():
    s = np.zeros((128, 512), np.float32)
    for w in range(4):
        s[2 * w, w * 128:w * 128 + 64] = 1.0
        s[2 * w + 1, w * 128 + 64:w * 128 + 128] = 1.0
    return s


def _selc():
    s = np.zeros((128, 64), np.float32)
    for i in range(8):
        s[64, 8 * i + i] = 1.0
    return s


def _to_sbuf_chunks(a, nch):
    """[nch*128, F] row-major -> [128, nch*F] SBUF-native layout."""
    n, fdim = a.shape
    assert n == nch * 128
    return np.ascontiguousarray(
        a.reshape(nch, 128, fdim).transpose(1, 0, 2).reshape(128, nch * fdim))


def _prep_core_inputs(b, g, x, Wq, bq, Wk, bk, Wv, bv, Wo, bo):
    f = np.float32
    xt = _to_sbuf_chunks(np.ascontiguousarray(x[b].T, dtype=f), NCH)
    def pack(W, bvec):
        cols = []
        bp = np.empty((2, 128, 1), f)
        for p in range(2):
            h0, h1 = 4 * g + 2 * p, 4 * g + 2 * p + 1
            Wp = np.concatenate([W[:, h0 * HD:(h0 + 1) * HD],
                                 W[:, h1 * HD:(h1 + 1) * HD]], axis=1)
            cols.append(_to_sbuf_chunks(np.ascontiguousarray(Wp, f), NCH))
            bp[p, 0:64, 0] = bvec[h0 * HD:(h0 + 1) * HD]
            bp[p, 64:128, 0] = bvec[h1 * HD:(h1 + 1) * HD]
        return np.concatenate(cols, axis=1), bp
    wq, bqp = pack(Wq, bq)
    wk, bkp = pack(Wk, bk)
    wv = _to_sbuf_chunks(np.ascontiguousarray(Wv[:, g * CLOC:(g + 1) * CLOC], f), NCH)
    wo = _to_sbuf_chunks(np.ascontiguousarray(Wo[g * CLOC:(g + 1) * CLOC, :], f), 2)
    return {"xt": xt, "wq": wq, "wk": wk, "wv": wv, "wo": wo,
            "bq": bqp, "bk": bkp,
            "ones": np.ones((128, NTT, NHL, 8), ml_dtypes.bfloat16),
            "sels": _sels(), "selc": _selc(),
            "tri": np.triu(np.ones((128, 128))).astype(ml_dtypes.bfloat16)}


def _run(inputs, trace=False, tmpdir=None):
    if "nc" not in _cache:
        _cache["nc"] = _build()
    nc = _cache["nc"]
    args = [np.asarray(inputs[k], np.float32) for k in
            ("x", "Wq", "bq", "Wk", "bk", "Wv", "bv", "Wo", "bo")]
    x, Wq, bq, Wk, bk, Wv, bv, Wo, bo = args
    in_maps = [_prep_core_inputs(c // 4, c % 4, x, Wq, bq, Wk, bk, Wv, bv, Wo, bo)
               for c in range(8)]
    res = bass_utils.run_bass_kernel_spmd(nc, in_maps, core_ids=list(range(8)),
                                          trace=trace, tmpdir=tmpdir)
    corr = (bv.astype(np.float64) @ Wo.astype(np.float64) + bo).astype(np.float32)
    out = np.empty((B, T, C), np.float32)
    for b in range(B):
        acc = np.zeros((T, C), np.float64)
        for g in range(4):
            acc += res.results[b * 4 + g]["out"]
        out[b] = (acc + corr).astype(np.float32)
    return out, res


def kernel(x, Wq, bq, Wk, bk, Wv, bv, Wo, bo):
    out, _ = _run(dict(x=x, Wq=Wq, bq=bq, Wk=Wk, bk=bk, Wv=Wv, bv=bv,
                       Wo=Wo, bo=bo))
    return out


def run_profiled(x, Wq, bq, Wk, bk, Wv, bv, Wo, bo, tmpdir=None):
    out, res = _run(dict(x=x, Wq=Wq, bq=bq, Wk=Wk, bk=bk, Wv=Wv, bv=bv,
                         Wo=Wo, bo=bo), trace=True, tmpdir=tmpdir)
    return out, res.exec_time_ns, res



# revision 4
# speedup vs baseline: 1.3270x; 1.3270x over previous
"""Causal self-attention kernel for Trainium2, 8-way sharded.

Problem: B=2, T=2048, C=1024, NH=16, hd=64. fp32 in/out.

Sharding: core = (batch b, head-group g of 4 heads). Each core computes its
4 heads' attention for its batch plus the partial output projection
y_local @ Wo[g*256:(g+1)*256, :]; the host sums the 4 partials per batch
(biases bv/bo are folded in exactly via a host-side correction row).

Device design (v2 — all-fp16, PE-bound, streamed):
  - Every matmul operand is fp16 (1 PE cycle/row at any free size, fp32 PSUM
    accumulate). Host-side sim: rel err ~4e-4 (vs 2e-2 budget).
  - Startup: pair-0 q/k projections run CHUNK-major (8 PSUM banks = 2 proj x
    4 windows) so the PE consumes each 0.5MB x-chunk as its DMA lands;
    no serial DMA wait.
  - Main loop is WINDOW-major: for each 512-wide tq window w: attention for
    head-pair 0, then pair 1, then the output projection of the window's 4
    t-tiles (streamed out in fp16; host upcasts) - no serial tail.
  - Scores S^T[tk,tq] = kt.T @ qt as K=64 matmuls; the two heads of a pair
    sit in PE row groups 0/64 and run concurrently (observed ~2x).
  - Diagonal-window trimming: for the diag tk-chunk jp (0-3), scores/exp/PV
    only touch tq columns >= 128*jp (~20% less score+exp+PV work); the
    128x128 diagonal block is masked with an upper-tri fp16 multiply.
  - Softmax without max-subtraction (S/8 is ~N(0,0.41), exp-safe).
  - PV: y^T[72,tq] += V_aug[tk,72].T @ P^T with 8 ones columns appended to V
    so denominators accumulate in PSUM row 64 for free; a K=1 selector
    matmul collects both heads' denominators into rows 0-1 of a per-window
    board; one 2-lane reciprocal + one K=2 broadcast matmul + one [128,512]
    vector multiply normalizes the window.
"""
import contextlib

import ml_dtypes
import numpy as np

import concourse.bass as bass
import concourse.tile as tile
from concourse import bacc, mybir
from concourse import bass_utils

bass_utils.upload_artifacts = lambda tmpdir: "local://skipped"

B, T, C = 2, 2048, 1024
NH, HD = 16, 64
NHL = 4            # heads per core
CLOC = NHL * HD    # 256 local channels
NCH = C // 128     # 8 contraction chunks
TQW = 512          # tq window
NW = T // TQW      # 4 windows
NTT = T // 128     # 16 t-tiles / tk-chunks
VSTR = HD + 8      # 72: v cols per head + 8 ones cols (denoms at row 64)
LAG = 2            # PV trails S^T by this many tk-chunks
F32 = mybir.dt.float32
F16 = mybir.dt.float16
EXP = mybir.ActivationFunctionType.Exp

_cache = {}


def _build():
    nc = bacc.Bacc("TRN2", target_bir_lowering=False, debug=False, num_devices=8)

    xt_ap = nc.dram_tensor("xt", [128, NCH * T], F16, kind="ExternalInput").ap()
    wq_ap = nc.dram_tensor("wq", [128, 2 * NCH * 128], F16, kind="ExternalInput").ap()
    wk_ap = nc.dram_tensor("wk", [128, 2 * NCH * 128], F16, kind="ExternalInput").ap()
    wv_ap = nc.dram_tensor("wv", [128, NCH * CLOC], F16, kind="ExternalInput").ap()
    wo_ap = nc.dram_tensor("wo", [128, 2 * C], F16, kind="ExternalInput").ap()
    bq_ap = nc.dram_tensor("bq", [2, 128, 1], F32, kind="ExternalInput").ap()
    bk_ap = nc.dram_tensor("bk", [2, 128, 1], F32, kind="ExternalInput").ap()
    ones_ap = nc.dram_tensor("ones", [128, NTT, NHL, 8], F16, kind="ExternalInput").ap()
    sels_ap = nc.dram_tensor("sels", [128, 128], F16, kind="ExternalInput").ap()
    selc_ap = nc.dram_tensor("selc", [128, 4], F16, kind="ExternalInput").ap()
    tri_ap = nc.dram_tensor("tri", [128, 128], F16, kind="ExternalInput").ap()
    out_ap = nc.dram_tensor("out", [T, C], F16, kind="ExternalOutput").ap()

    with tile.TileContext(nc) as tc, contextlib.ExitStack() as ctx:
        sb = ctx.enter_context(tc.tile_pool(name="sb", bufs=1))
        r_pool = ctx.enter_context(tc.tile_pool(name="rp", bufs=1))
        ost_pool = ctx.enter_context(tc.tile_pool(name="ost", bufs=4))
        pt_pool = ctx.enter_context(tc.tile_pool(name="ptp", bufs=10))
        ps = ctx.enter_context(tc.tile_pool(name="ps", bufs=1, space="PSUM"))

        # ---- persistent SBUF tensors ----
        wqs = sb.tile([128, 2 * NCH * 128], F16, tag="wqs")
        wks = sb.tile([128, 2 * NCH * 128], F16, tag="wks")
        wvs = sb.tile([128, NCH * CLOC], F16, tag="wvs")
        wos = sb.tile([128, 2 * C], F16, tag="wos")
        xts = sb.tile([128, NCH * T], F16, tag="xts")
        qts = [sb.tile([128, T], F16, tag=f"qt{p}", name=f"qt{p}") for p in range(2)]
        kts = [sb.tile([128, T], F16, tag=f"kt{p}", name=f"kt{p}") for p in range(2)]
        vna = sb.tile([128, NTT * NHL * VSTR], F16, tag="vna")
        yts = [sb.tile([128, T], F16, tag=f"yt{p}", name=f"yt{p}") for p in range(2)]
        bqs = [sb.tile([128, 1], F32, tag=f"bq{p}", name=f"bqs{p}") for p in range(2)]
        bks = [sb.tile([128, 1], F32, tag=f"bk{p}", name=f"bks{p}") for p in range(2)]
        sels = sb.tile([128, 128], F16, tag="sels")
        selc = sb.tile([128, 4], F16, tag="selc")
        dsb = sb.tile([128, TQW], F32, tag="dsb")
        rc16 = sb.tile([128, TQW], F16, tag="rc16")
        rcoll = sb.tile([128, TQW], F32, tag="rcoll")
        tri = sb.tile([128, 128], F16, tag="tri")

        vna4 = vna[:].rearrange("p (t h v) -> p t h v", t=NTT, h=NHL)
        HNCH = NCH * 128  # cols per pair in wq/wk

        # ---- input DMAs: 3 queues, ordered so phase A streams chunk-major.
        # PE wants chunk c at ~(6 + 3.4c)us; per-queue ~0.5MB / 5us.
        nc.sync.dma_start(wqs[:, 0:HNCH], wq_ap[:, 0:HNCH])
        nc.gpsimd.dma_start(wks[:, 0:HNCH], wk_ap[:, 0:HNCH])
        chq = {0: nc.scalar, 1: nc.sync, 2: nc.gpsimd, 3: nc.gpsimd,
               4: nc.scalar, 5: nc.sync, 6: nc.gpsimd, 7: nc.scalar}
        order = [0, 1, 2, 3, 4, 5, 6, 7]
        for c in order:
            chq[c].dma_start(xts[:, c * T:(c + 1) * T], xt_ap[:, c * T:(c + 1) * T])
        for p in range(2):
            nc.sync.dma_start(bqs[p][:], bq_ap[p])
            nc.sync.dma_start(bks[p][:], bk_ap[p])
        nc.sync.dma_start(sels[:], sels_ap[:])
        nc.sync.dma_start(selc[:], selc_ap[:])
        nc.sync.dma_start(tri[:], tri_ap[:])
        nc.gpsimd.dma_start(wvs[:], wv_ap[:])
        nc.gpsimd.dma_start(wks[:, HNCH:], wk_ap[:, HNCH:])
        nc.gpsimd.dma_start(vna4[:, :, :, HD:HD + 8], ones_ap[:])
        nc.scalar.dma_start(wqs[:, HNCH:], wq_ap[:, HNCH:])
        nc.scalar.dma_start(wos[:], wo_ap[:])

        pts = {}
        D_tiles = {}

        # ---------- emission pieces ----------
        def warm():
            wtile = sb.tile([128, 640], F16, tag="warm")
            wjunk = sb.tile([128, 8], F32, tag="wjunk")
            nc.vector.memset(wtile[:], 0.0)
            wp = ps.tile([128, TQW], F32, tag="ka0", bufs=1, name="warm_ps")
            for _ in range(10):
                nc.tensor.matmul(wp[:], wtile[:, 0:128], wtile[:, 128:640],
                                 start=True, stop=True)
            nc.vector.tensor_copy(wjunk[:], wp[:, 0:8])

        def qk_p0_streamed():
            qacc = [ps.tile([128, TQW], F32, tag=f"qa{w}", bufs=1,
                            name=f"qacc{w}") for w in range(NW)]
            kacc = [ps.tile([128, TQW], F32, tag=f"ka{w}", bufs=1,
                            name=f"kacc{w}") for w in range(NW)]
            for c in range(NCH):
                for w in range(NW):
                    mv = xts[:, c * T + w * TQW: c * T + (w + 1) * TQW]
                    nc.tensor.matmul(qacc[w][:], wqs[:, c * 128:(c + 1) * 128],
                                     mv, start=(c == 0), stop=(c == NCH - 1))
                    nc.tensor.matmul(kacc[w][:], wks[:, c * 128:(c + 1) * 128],
                                     mv, start=(c == 0), stop=(c == NCH - 1))
            for w in range(NW):
                nc.vector.tensor_scalar_add(
                    qts[0][:, w * TQW:(w + 1) * TQW], qacc[w][:], bqs[0][:])
                nc.vector.tensor_scalar_add(
                    kts[0][:, w * TQW:(w + 1) * TQW], kacc[w][:], bks[0][:])

        def qk_p1():
            i = 0
            for wsb, dst, bias in ((wqs, qts[1], bqs[1]), (wks, kts[1], bks[1])):
                for w in range(NW):
                    acc = ps.tile([128, TQW], F32, tag=("qa0", "qa1")[i % 2],
                                  bufs=1, name=f"qk1acc{i}")
                    i += 1
                    for c in range(NCH):
                        nc.tensor.matmul(
                            acc[:], wsb[:, HNCH + c * 128: HNCH + (c + 1) * 128],
                            xts[:, c * T + w * TQW: c * T + (w + 1) * TQW],
                            start=(c == 0), stop=(c == NCH - 1))
                    nc.vector.tensor_scalar_add(dst[:, w * TQW:(w + 1) * TQW],
                                                acc[:], bias[:])

        def v_tile(tt):
            acc = ps.tile([128, CLOC], F32, tag="ka1", bufs=1, name=f"v{tt}")
            for c in range(NCH):
                nc.tensor.matmul(acc[:],
                                 xts[:, c * T + tt * 128: c * T + tt * 128 + 128],
                                 wvs[:, c * CLOC:(c + 1) * CLOC],
                                 start=(c == 0), stop=(c == NCH - 1))
            nc.scalar.copy(vna4[:, tt, :, 0:HD],
                           acc[:].rearrange("p (h d) -> p h d", h=NHL))

        def st_slot(p, w, c):
            jp = c - 4 * w
            o = 128 * jp if jp > 0 else 0
            for h in range(2):
                st = ps.tile([128, TQW], F32, tag=("qa2", "qa3")[h], bufs=1,
                             name=f"st{p}{w}{c}{h}")
                nc.tensor.matmul(
                    st[:, o:TQW],
                    kts[p][h * 64:(h + 1) * 64, c * 128:(c + 1) * 128],
                    qts[p][h * 64:(h + 1) * 64, w * TQW + o:(w + 1) * TQW],
                    start=True, stop=True)
                pt = pt_pool.tile([128, TQW], F16, tag="pt", name=f"pt{p}{w}{c}{h}")
                nc.scalar.activation(pt[:, o:TQW], st[:, o:TQW], EXP, scale=0.125)
                if jp >= 0:
                    dslc = pt[:, o:o + 128]
                    nc.vector.tensor_mul(dslc, dslc, tri[:])
                pts[(p, w, c, h)] = pt

        def pv_chunk(p, w, c, accs):
            nchunks = 4 * (w + 1)
            jp = c - 4 * w
            o = 128 * jp if jp > 0 else 0
            for h in range(2):
                pt = pts.pop((p, w, c, h))
                vb = c * NHL * VSTR + (2 * p + h) * VSTR
                nc.tensor.matmul(
                    accs[h][0:VSTR, o:TQW],
                    vna[:, vb:vb + VSTR],
                    pt[:, o:TQW],
                    start=(c == 0), stop=(c == nchunks - 1),
                    skip_group_check=True)

        def att_window(p, w, mid_hook=None):
            accs = [ps.tile([128, TQW], F32, tag=("ka2", "ka3")[h], bufs=1,
                            name=f"acc{p}{w}{h}") for h in range(2)]
            nchunks = 4 * (w + 1)
            for g in range(nchunks + LAG):
                if g < nchunks:
                    st_slot(p, w, g)
                if g == 1 and mid_hook is not None:
                    mid_hook()
                if g >= LAG:
                    pv_chunk(p, w, g - LAG, accs)
            D = ps.tile([128, TQW], F32, tag="ka0", bufs=1, name=f"D{p}{w}")
            D_tiles[(p, w)] = D
            for h in range(2):
                nc.vector.tensor_copy(yts[p][h * 64:(h + 1) * 64,
                                             w * TQW:(w + 1) * TQW],
                                      accs[h][0:HD, :])
                rstage = r_pool.tile([128, TQW], F16, tag="rstage", bufs=2,
                                     name=f"rst{p}{w}{h}")
                nc.vector.tensor_copy(rstage[HD:HD + 1, :], accs[h][HD:HD + 1, :])
                nc.tensor.matmul(D[0:2, :], selc[HD:HD + 1, 2 * h:2 * h + 2],
                                 rstage[HD:HD + 1, :], start=(h == 0),
                                 stop=(h == 1), skip_group_check=True)

        def norm(p, w):
            D = D_tiles.pop((p, w))
            nc.vector.tensor_copy(dsb[0:2, :], D[0:2, :])
            nc.vector.reciprocal(rcoll[0:2, :], dsb[0:2, :])
            nc.vector.tensor_copy(rc16[0:2, :], rcoll[0:2, :])
            R = ps.tile([128, TQW], F32, tag="ka0", bufs=1, name=f"R{p}{w}")
            nc.tensor.matmul(R[:], sels[0:2, 0:128], rc16[0:2, :],
                             start=True, stop=True)
            yslc = yts[p][:, w * TQW:(w + 1) * TQW]
            nc.vector.tensor_mul(yslc, yslc, R[:])

        odma = [nc.sync, nc.gpsimd, nc.scalar]

        def out_tile(tt):
            for nh in range(2):
                po = ps.tile([128, TQW], F32, tag=("qa0", "qa1")[nh], bufs=1,
                             name=f"po{tt}{nh}")
                for cc in range(2):
                    nc.tensor.matmul(
                        po[:], yts[cc][:, tt * 128:(tt + 1) * 128],
                        wos[:, cc * C + nh * TQW: cc * C + nh * TQW + TQW],
                        start=(cc == 0), stop=(cc == 1))
                ost = ost_pool.tile([128, TQW], F16, tag="ost", name=f"o{tt}{nh}")
                if nh == 0:
                    nc.vector.tensor_copy(ost[:], po[:])
                else:
                    nc.scalar.copy(ost[:], po[:])
                odma[(2 * tt + nh) % 3].dma_start(
                    out_ap[tt * 128:(tt + 1) * 128, nh * TQW:(nh + 1) * TQW],
                    ost[:])

        # ---------- schedule ----------
        warm()
        qk_p0_streamed()
        qk_p1()
        for tt in range(4):
            v_tile(tt)
        for w in range(NW):
            att_window(0, w)
            att_window(1, w, mid_hook=lambda w=w: norm(0, w))
            # interleave next window's v tiles with the norm chain + out
            nxt = [4 * (w + 1) + i for i in range(4)] if w < NW - 1 else []
            if nxt:
                v_tile(nxt[0])
            norm(1, w)
            if nxt:
                v_tile(nxt[1])
            for i, tt in enumerate(range(4 * w, 4 * w + 4)):
                out_tile(tt)
                if i < len(nxt) - 2:
                    v_tile(nxt[i + 2])

    nc.compile()
    return nc


def _sels():
    s = np.zeros((128, 128), np.float16)
    s[0, 0:64] = 1.0
    s[1, 64:128] = 1.0
    return s


def _selc():
    s = np.zeros((128, 4), np.float16)
    s[64, 0] = 1.0   # h0 -> D row 0
    s[64, 3] = 1.0   # h1 -> D row 1
    return s


def _to_sbuf_chunks(a, nch, dt=np.float16):
    """[nch*128, F] row-major -> [128, nch*F] SBUF-native layout."""
    n, fdim = a.shape
    assert n == nch * 128
    return np.ascontiguousarray(
        a.reshape(nch, 128, fdim).transpose(1, 0, 2).reshape(128, nch * fdim)
    ).astype(dt)


def _prep_core_inputs(b, g, x, Wq, bq, Wk, bk, Wv, bv, Wo, bo):
    f = np.float32
    xt = _to_sbuf_chunks(np.ascontiguousarray(x[b].T, dtype=f), NCH)
    def pack(W, bvec):
        cols = []
        bp = np.empty((2, 128, 1), f)
        for p in range(2):
            h0, h1 = 4 * g + 2 * p, 4 * g + 2 * p + 1
            Wp = np.concatenate([W[:, h0 * HD:(h0 + 1) * HD],
                                 W[:, h1 * HD:(h1 + 1) * HD]], axis=1)
            cols.append(_to_sbuf_chunks(np.ascontiguousarray(Wp, f), NCH))
            bp[p, 0:64, 0] = bvec[h0 * HD:(h0 + 1) * HD]
            bp[p, 64:128, 0] = bvec[h1 * HD:(h1 + 1) * HD]
        return np.concatenate(cols, axis=1), bp
    wq, bqp = pack(Wq, bq)
    wk, bkp = pack(Wk, bk)
    wv = _to_sbuf_chunks(np.ascontiguousarray(Wv[:, g * CLOC:(g + 1) * CLOC], f), NCH)
    wo = _to_sbuf_chunks(np.ascontiguousarray(Wo[g * CLOC:(g + 1) * CLOC, :], f), 2)
    return {"xt": xt, "wq": wq, "wk": wk, "wv": wv, "wo": wo,
            "bq": bqp, "bk": bkp,
            "ones": np.ones((128, NTT, NHL, 8), np.float16),
            "sels": _sels(), "selc": _selc(),
            "tri": np.triu(np.ones((128, 128))).astype(np.float16)}


def _run(inputs, trace=False, tmpdir=None):
    if "nc" not in _cache:
        _cache["nc"] = _build()
    nc = _cache["nc"]
    args = [np.asarray(inputs[k], np.float32) for k in
            ("x", "Wq", "bq", "Wk", "bk", "Wv", "bv", "Wo", "bo")]
    x, Wq, bq, Wk, bk, Wv, bv, Wo, bo = args
    in_maps = [_prep_core_inputs(c // 4, c % 4, x, Wq, bq, Wk, bk, Wv, bv, Wo, bo)
               for c in range(8)]
    res = bass_utils.run_bass_kernel_spmd(nc, in_maps, core_ids=list(range(8)),
                                          trace=trace, tmpdir=tmpdir)
    corr = (bv.astype(np.float64) @ Wo.astype(np.float64) + bo).astype(np.float64)
    out = np.empty((B, T, C), np.float32)
    for b in range(B):
        acc = np.zeros((T, C), np.float64)
        for g in range(4):
            acc += res.results[b * 4 + g]["out"].astype(np.float64)
        out[b] = (acc + corr).astype(np.float32)
    return out, res


def kernel(x, Wq, bq, Wk, bk, Wv, bv, Wo, bo):
    out, _ = _run(dict(x=x, Wq=Wq, bq=bq, Wk=Wk, bk=bk, Wv=Wv, bv=bv,
                       Wo=Wo, bo=bo))
    return out


def run_profiled(x, Wq, bq, Wk, bk, Wv, bv, Wo, bo, tmpdir=None):
    out, res = _run(dict(x=x, Wq=Wq, bq=bq, Wk=Wk, bk=bk, Wv=Wv, bv=bv,
                         Wo=Wo, bo=bo), trace=True, tmpdir=tmpdir)
    return out, res.exec_time_ns, res
